# revision 23
# baseline (speedup 1.0000x reference)
"""Trainium2 Bass kernel for nn_ContrastiveLoss (N=384, D=128, 8 cores).

Label-structure reduction (exact re-grouping of the reference math):
  y = concat(targets, targets) has every label twice (rows i and i+192).
  Pairwise label distance a[i,j] = |y_i - y_j| therefore lives on a 192x192
  label grid; the per-anchor comparison mask cp[v,P] = [a_v < a_P] depends
  only on the anchor's label row. Columns p and p+192 of denom are equal, and
  j and j+192 contributions pair-reduce. With
     w[i,j] = exp(-dist(z_i,z_j)/TEMP) * sigmoid(TAU*a) * [j != i]
     m[i,v] = coef[v,i] * (w[i,j=v] + w[i,j=v+192]),
     coef = (POS_W-1)*[t_v > y_i] - NEG_W*[t_v <= y_i]
     base_i = sum_j w[i,j]
  denomL[i,P] = base_i + sum_v cp_i[v,P] * m[i,v]        (192 P columns)
  sum_{p!=i} log denom[i,p] = 2*sum_P log denomL[i,P] - log base_i
  loss = -(sum_i sum_{p!=i} s[i,p] - sum_i sum_{p!=i} log denom) / (N(N-1)).

Device work per core (24 label rows = 48 anchor rows):
  phase 1: Gram matmuls -> dist -> exp (ln/exp chain, one ACT table set),
  w tiles, pair-reduce, m weights (bf16, zero-padded to M=32 col groups).
  phase 2: per PSUM bank, 4 col-tiled (tile_position) matmul groups compute
  denomL for 4 row-pairs directly in PSUM (cp ships as host fp8 with a
  bias row carrying base_i), then ONE scalar Ln(+eps, accum_out) per bank
  fuses log + the P-reduction. cp is label-only data, precomputed on host.
"""

import os
import sys

import numpy as np

for _p in ("/opt/trn_rl_repo", "/root/.axon_site/_ro/trn_rl_repo"):
    if os.path.isdir(_p) and _p not in sys.path:
        sys.path.insert(0, _p)

import ml_dtypes

import concourse.bass as bass
import concourse.bacc as bacc
import concourse.mybir as mybir
from concourse import tile
from concourse.bass_utils import run_bass_kernel_spmd

F32 = mybir.dt.float32
BF16 = mybir.dt.bfloat16
FP8 = mybir.dt.float8e4
AF = mybir.ActivationFunctionType
OP = mybir.AluOpType

B = 192
N = 2 * B
D = 128
NC = 8
V = 192           # labels
NP = 24           # label rows (pairs) per core
RW = 2 * NP       # 48 anchor rows per core
PB = 192          # P columns per pair block
NB = 3            # PSUM banks in phase 2 (8 pairs each: 4 col groups x 2 col sets)
NG = 4            # col groups per bank

TEMP = 2.0
TAU = 1.0
POS_W = 0.1
NEG_W = 1.0

# packed_bf column layout (bf16, [128, PW])
C_ZT2 = 0          # 0:384    -2*z~ transposed (j natural order)
C_ZOWN = 384       # 384:432  z~ own cols (2p -> row 24k+p, 2p+1 -> +192)
C_ZOWNX = 432      # 432:480  -2*z~ own cols
C_DWND = 480       # 480:672  dwnd per chunk slice (48 cols each: ch0,ch2,ch1,ch3)
C_COEF = 672       # 672:768  coef0 (128 rows) | coef1 (64 rows)
PW = 768

# j chunks: (z-col slice, Pc, SQ/DWND col slice index)
CHUNKS = [
    (0, 128, 0),      # ch0: j 0:128     v 0:128   copy 1
    (192, 128, 1),    # ch2: j 192:320   v 0:128   copy 2
    (128, 64, 2),     # ch1: j 128:192   v 128:192 copy 1
    (320, 64, 3),     # ch3: j 320:384   v 128:192 copy 2
]


def _build_program():
    # Force the single ACT table set containing Ln+Exp+Copy (set 6,
    # natural_log_exp_and_others): the greedy table chooser otherwise picks
    # exp_and_others for Exp and natural_log for Ln, thrashing 4 loads
    # (1.5us each). Emptying every other set makes set 6 the unique choice;
    # ids stay aligned with act_info.json. Patch is scoped to this build.
    import concourse.bacc as _bacc_mod
    _orig_gat = _bacc_mod.get_activation_tables

    def _gat(arch):
        tables = _orig_gat(arch)
        return {
            name: (funcs if name == "natural_log_exp_and_others" else set())
            for name, funcs in tables.items()
        }

    _bacc_mod.get_activation_tables = _gat
    try:
        return _build_program_inner()
    finally:
        _bacc_mod.get_activation_tables = _orig_gat


def _build_program_inner():
    nc = bacc.Bacc("TRN2", target_bir_lowering=False, debug=False, num_devices=NC)

    packed = nc.dram_tensor("packed", [128, PW], BF16, kind="ExternalInput").ap()
    cpa_d = nc.dram_tensor("cpa", [128, NP * PB], FP8, kind="ExternalInput").ap()
    cpb_d = nc.dram_tensor("cpb", [65, NP * PB], FP8, kind="ExternalInput").ap()
    out_acc = nc.dram_tensor("acc", [1, 2 * RW + 2 * NB], F32, kind="ExternalOutput").ap()

    with tile.TileContext(nc) as tc:
        with (
            tc.tile_pool(name="sb", bufs=1) as sb,
            tc.tile_pool(name="lnp", bufs=2) as lnp,
        ):
            # ---------------- DMAs ----------------
            # packed in two regions so the Gram/zsq work starts before the
            # dwnd/coef half lands
            pk = sb.tile([128, PW], BF16, tag="pk")
            nc.sync.dma_start(pk[:, 0:C_DWND], packed[:, 0:C_DWND])
            nc.scalar.dma_start(pk[:, C_DWND:PW], packed[:, C_DWND:PW])
            cpa = sb.tile([128, NP * PB], FP8, tag="cpa")
            cpb = sb.tile([65, NP * PB], FP8, tag="cpb")
            for b in range(NB):
                sl = slice(b * 8 * PB, (b + 1) * 8 * PB)
                eng = nc.sync if b % 2 == 0 else nc.scalar
                eng.dma_start(cpa[:, sl], cpa_d[:, sl])
                eng2 = nc.scalar if b % 2 == 0 else nc.sync
                eng2.dma_start(cpb[:, sl], cpb_d[:, sl])

            # ---------------- consts ----------------
            quart = sb.tile([128, 1], BF16, tag="quart")
            nc.vector.memset(quart[:], 0.25)
            ones_b = sb.tile([128, 1], BF16, tag="ones_b")
            nc.vector.memset(ones_b[:], 1.0)
            ones_f = sb.tile([128, 1], F32, tag="ones_f")
            nc.vector.memset(ones_f[:], 1.0)
            onesrow = sb.tile([1, 128], BF16, tag="onesrow")
            nc.vector.memset(onesrow[:], 1.0)
            epscol = sb.tile([128, 1], F32, tag="eps")
            nc.vector.memset(epscol[:], 1e-30)
            vmask = sb.tile([128, 1], F32, tag="vmask")
            nc.vector.memset(vmask[:], 0.0)
            for q in range(NG):
                nc.vector.memset(vmask[32 * q : 32 * q + 2, :], 1.0)
            scr = sb.tile([128, 1], F32, tag="scr")
            # first scalar op: force-load the ln/exp ACT table set early
            nc.scalar.activation(scr[:], epscol[:], AF.Exp)

            mA = sb.tile([128, NP * 32], BF16, tag="mA")
            nc.gpsimd.memset(mA[:], 0.0)
            mB = sb.tile([65, NP * 32], BF16, tag="mB")
            nc.gpsimd.memset(mB[:], 0.0)
            # sqm pre-filled with 1.0: rows 64:128 of the 64-partition chunk
            # slices stay 1.0 -> ln 0, dist 1, wnd = e*0 = 0 (no NaNs)
            sqm = sb.tile([128, 4 * RW], F32, tag="sqm")
            nc.gpsimd.memset(sqm[:], 1.0)

            # ---------------- phase 1 ----------------
            zsq4 = sb.tile([128, N], BF16, tag="zsq4")
            nc.vector.tensor_tensor(
                zsq4[:], pk[:, C_ZT2 : C_ZT2 + N], pk[:, C_ZT2 : C_ZT2 + N],
                op=OP.mult,
            )
            zsqo4 = sb.tile([128, RW], BF16, tag="zsqo4")
            nc.gpsimd.tensor_tensor(
                zsqo4[:], pk[:, C_ZOWNX : C_ZOWNX + RW],
                pk[:, C_ZOWNX : C_ZOWNX + RW], op=OP.mult,
            )

            with (
                tc.tile_pool(name="ps_own", bufs=1, space="PSUM") as ps_own,
                tc.tile_pool(name="ps_g", bufs=1, space="PSUM") as ps_g,
                tc.tile_pool(name="ps_acc", bufs=1, space="PSUM") as ps_acc,
                tc.tile_pool(name="ps_acd", bufs=1, space="PSUM") as ps_acd,
            ):
                # n2own row [1,48] and n2col row [1,384] (0.25 * colsums of 4z^2)
                n2own_ps = ps_own.tile([1, RW], F32, tag="own")
                nc.tensor.matmul(
                    n2own_ps[:], quart[:], zsqo4[:], start=True, stop=True
                )
                n2row_ps = ps_own.tile([1, N], F32, tag="n2r")
                for zc, pc in ((0, 128), (192, 128), (128, 64), (320, 64)):
                    nc.tensor.matmul(
                        n2row_ps[0:1, zc : zc + pc], quart[:],
                        zsq4[:, zc : zc + pc], start=True, stop=True,
                        skip_group_check=True,
                    )
                n2own_s = sb.tile([1, RW], BF16, tag="n2own_s")
                nc.vector.tensor_copy(n2own_s[:], n2own_ps[:])
                n2row_s = sb.tile([1, N], BF16, tag="n2row_s")
                for zc, pc in ((0, 128), (192, 128), (128, 64), (320, 64)):
                    nc.vector.tensor_copy(
                        n2row_s[0:1, zc : zc + pc], n2row_ps[0:1, zc : zc + pc]
                    )

                lsq = sb.tile([128, 4 * RW], F32, tag="lsq")
                dist = sb.tile([128, 4 * RW], F32, tag="dist")
                e_t = sb.tile([128, 4 * RW], BF16, tag="e")
                wnd = sb.tile([128, 4 * RW], BF16, tag="wnd")
                acc_ps = ps_acc.tile([1, RW], F32, tag="acc")
                acd_ps = ps_acd.tile([1, RW], F32, tag="acd")

                # halves: A = ch0+ch2 (128 rows), B = ch1+ch3 (64 rows)
                HALVES = [(0, 128, (0, 192)), (1, 64, (128, 320))]
                acc_first = True
                for hi, pc, zcs in HALVES:
                    hsl = slice(hi * 2 * RW, (hi + 1) * 2 * RW)
                    gb = ps_g.tile([pc, 2 * RW], F32, tag=f"gb{hi}")
                    for ci, zc in enumerate(zcs):
                        csl = slice(ci * RW, (ci + 1) * RW)
                        nc.tensor.matmul(
                            gb[:, csl], pk[:, C_ZT2 + zc : C_ZT2 + zc + pc],
                            pk[:, C_ZOWN : C_ZOWN + RW],
                            start=(ci == 0), stop=False,
                            skip_group_check=True,
                        )
                    for ci, zc in enumerate(zcs):
                        csl = slice(ci * RW, (ci + 1) * RW)
                        nc.tensor.matmul(
                            gb[:, csl], onesrow[0:1, 0:pc], n2own_s[:],
                            start=False, stop=False, skip_group_check=True,
                        )
                        nc.tensor.matmul(
                            gb[:, csl], n2row_s[0:1, zc : zc + pc],
                            onesrow[0:1, 0:RW],
                            start=False, stop=(ci == 1), skip_group_check=True,
                        )
                    nc.vector.tensor_scalar(
                        sqm[0:pc, hsl], gb[:], 0.0, None, op0=OP.max
                    )
                    nc.scalar.activation(
                        lsq[:, hsl], sqm[:, hsl], AF.Ln, bias=epscol[:]
                    )
                    nc.scalar.activation(
                        dist[:, hsl], lsq[:, hsl], AF.Exp, scale=0.5
                    )
                    nc.scalar.activation(
                        e_t[:, hsl], dist[:, hsl], AF.Exp, scale=-1.0 / TEMP
                    )
                    nc.vector.tensor_tensor(
                        wnd[:, hsl], e_t[:, hsl],
                        pk[:, C_DWND + hi * 2 * RW : C_DWND + (hi + 1) * 2 * RW],
                        op=OP.mult,
                    )
                    # column sums for this half (2 wnd + 2 dist chunk slices)
                    for ci in range(2):
                        sl2 = slice((2 * hi + ci) * RW, (2 * hi + ci + 1) * RW)
                        nc.tensor.matmul(
                            acc_ps[0:1, 0:RW], ones_b[0:pc, :], wnd[0:pc, sl2],
                            start=(hi == 0 and ci == 0),
                            stop=(hi == 1 and ci == 1),
                            skip_group_check=True,
                        )
                    for ci in range(2):
                        sl2 = slice((2 * hi + ci) * RW, (2 * hi + ci + 1) * RW)
                        nc.tensor.matmul(
                            acd_ps[0:1, 0:RW], ones_f[0:pc, :],
                            dist[0:pc, sl2],
                            start=(hi == 0 and ci == 0),
                            stop=(hi == 1 and ci == 1),
                            skip_group_check=True,
                        )
                    # pair-reduce + m weights for this half
                    if hi == 0:
                        wr0 = sb.tile([128, RW], BF16, tag="wr0")
                        nc.vector.tensor_tensor(
                            wr0[:], wnd[:, 0:RW], wnd[:, RW : 2 * RW], op=OP.add
                        )
                        mA_sc = mA[:].rearrange(
                            "p (n s) -> p n s", n=NP, s=32
                        )[:, :, 0:2]
                        nc.vector.tensor_tensor(
                            mA_sc,
                            wr0[:].rearrange("p (n r) -> p n r", n=NP, r=2),
                            pk[:, C_COEF : C_COEF + RW].rearrange(
                                "p (n r) -> p n r", n=NP, r=2
                            ),
                            op=OP.mult,
                        )
                    else:
                        wr1 = sb.tile([64, RW], BF16, tag="wr1")
                        nc.vector.tensor_tensor(
                            wr1[:], wnd[0:64, 2 * RW : 3 * RW],
                            wnd[0:64, 3 * RW : 4 * RW], op=OP.add
                        )
                        mB_sc = mB[0:64].rearrange(
                            "p (n s) -> p n s", n=NP, s=32
                        )[:, :, 0:2]
                        nc.vector.tensor_tensor(
                            mB_sc,
                            wr1[:].rearrange("p (n r) -> p n r", n=NP, r=2),
                            pk[0:64, C_COEF + RW : C_COEF + 2 * RW].rearrange(
                                "p (n r) -> p n r", n=NP, r=2
                            ),
                            op=OP.mult,
                        )

                # bias row straight from PSUM on the vector engine
                mBias_sc = mB[64:65].rearrange(
                    "p (n s) -> p n s", n=NP, s=32
                )[:, :, 0:2]
                nc.vector.tensor_copy(
                    mBias_sc,
                    acc_ps[0:1, 0:RW].rearrange(
                        "p (n r) -> p n r", n=NP, r=2
                    ),
                )
                accsb = sb.tile([1, 2 * RW + 2 * NB], F32, tag="accsb")
                nc.vector.tensor_copy(accsb[0:1, 0:RW], acc_ps[:])
                nc.vector.tensor_copy(accsb[0:1, RW : 2 * RW], acd_ps[:])

            # ---------------- phase 2 ----------------
            red = sb.tile([128, 2 * NB], F32, tag="red")
            with tc.tile_pool(name="ps_den", bufs=3, space="PSUM") as ps_den:
                for b in range(NB):
                    den = ps_den.tile([128, 2 * PB], F32, tag="den")
                    for cs in range(2):
                        for q in range(NG):
                            p = 8 * b + 4 * cs + q
                            osl = den[
                                32 * q : 32 * q + 32, PB * cs : PB * (cs + 1)
                            ]
                            nc.tensor.matmul(
                                osl,
                                mA[:, 32 * p : 32 * p + 32],
                                cpa[:, PB * p : PB * (p + 1)],
                                start=(cs == 0), stop=False,
                                skip_group_check=True,
                                tile_position=(0, 32 * q),
                            )
                        for q in range(NG):
                            p = 8 * b + 4 * cs + q
                            osl = den[
                                32 * q : 32 * q + 32, PB * cs : PB * (cs + 1)
                            ]
                            nc.tensor.matmul(
                                osl,
                                mB[:, 32 * p : 32 * p + 32],
                                cpb[:, PB * p : PB * (p + 1)],
                                start=False, stop=(cs == 1),
                                skip_group_check=True,
                                tile_position=(0, 32 * q),
                            )
                        lnden = lnp.tile([128, PB], BF16, tag="ln")
                        nc.scalar.activation(
                            lnden[:], den[:, PB * cs : PB * (cs + 1)],
                            AF.Ln, bias=epscol[:],
                            accum_out=red[:, 2 * b + cs : 2 * b + cs + 1],
                        )
                # collapse the sparse red columns to 6 totals on the PE and
                # append to the single-row output (one DMA descriptor)
                redtot_ps = ps_den.tile([1, 2 * NB], F32, tag="rt")
                nc.tensor.matmul(
                    redtot_ps[:], vmask[:], red[:], start=True, stop=True
                )
                nc.vector.tensor_copy(accsb[0:1, 2 * RW :], redtot_ps[:])
            nc.sync.dma_start(out_acc, accsb[:])

    nc.compile()
    return nc


_NC_CACHE = None


def _get_nc():
    global _NC_CACHE
    if _NC_CACHE is None:
        _NC_CACHE = _build_program()
    return _NC_CACHE


def _make_in_maps(embeddings, targets):
    emb = np.ascontiguousarray(np.asarray(embeddings, dtype=np.float32))
    tgt = np.ascontiguousarray(np.asarray(targets, dtype=np.float32))
    z = emb.transpose(1, 0, 2).reshape(N, D)
    zb = z.astype(ml_dtypes.bfloat16)              # device z values
    t = tgt[:, 0]                                  # 192 labels (fp32)
    y = np.concatenate([t, t])                     # 384

    in_maps = []
    for core in range(NC):
        labs = np.arange(NP * core, NP * (core + 1))       # label rows
        own = np.empty(RW, np.int64)
        own[0::2] = labs
        own[1::2] = labs + B
        yo = y[own]                                        # [48]

        pkt = np.zeros((128, PW), np.float32)
        pkt[:, C_ZT2 : C_ZT2 + N] = -2.0 * zb.T.astype(np.float32)
        pkt[:, C_ZOWN : C_ZOWN + RW] = zb.T[:, own].astype(np.float32)
        pkt[:, C_ZOWNX : C_ZOWNX + RW] = -2.0 * zb.T[:, own].astype(np.float32)

        # dwnd = sigmoid(TAU*|y_i - y_j|) * [j != own_row], chunk layout
        a_all = np.abs(yo[None, :] - y[:, None])           # [384, 48] (j, i)
        dw = 1.0 / (1.0 + np.exp(-TAU * a_all))
        ndm = (np.arange(N)[:, None] != own[None, :]).astype(np.float32)
        dwnd = (dw * ndm).astype(np.float32)
        for (zc, pc, si) in CHUNKS:
            pkt[0:pc, C_DWND + si * RW : C_DWND + (si + 1) * RW] = dwnd[
                zc : zc + pc, :
            ]

        # coef[v, i] = (POS_W-1) if t_v > y_i else -NEG_W
        same = (t[:, None] > yo[None, :])
        coef = np.where(same, POS_W - 1.0, -NEG_W).astype(np.float32)
        pkt[:, C_COEF : C_COEF + RW] = coef[0:128]
        pkt[0:64, C_COEF + RW : C_COEF + 2 * RW] = coef[128:192]

        # cp blocks (exact fp32 label comparisons)
        cpa = np.zeros((128, NP * PB), np.float32)
        cpb = np.zeros((65, NP * PB), np.float32)
        for p, L in enumerate(labs):
            al = np.abs(t[L] - t)                          # [192]
            cp = (al[:, None] < al[None, :]).astype(np.float32)
            cpa[:, PB * p : PB * (p + 1)] = cp[0:128]
            cpb[0:64, PB * p : PB * (p + 1)] = cp[128:192]
        cpb[64, :] = 1.0

        in_maps.append({
            "packed": pkt.astype(ml_dtypes.bfloat16),
            "cpa": cpa.astype(ml_dtypes.float8_e4m3fn),
            "cpb": cpb.astype(ml_dtypes.float8_e4m3fn),
        })
    return in_maps


def _reduce_outs(outs_list):
    tot_s = 0.0
    tot_logd = 0.0
    for o in outs_list:
        acc = np.asarray(o["acc"], dtype=np.float64)[0]    # [99]
        base = acc[0:RW]
        dsum = acc[RW : 2 * RW]
        redtot = acc[2 * RW : 2 * RW + 2 * NB]
        tot_s += -dsum.sum() / TEMP
        tot_logd += 2.0 * redtot.sum() - np.log(base).sum()
    loss = -(tot_s - tot_logd) / (N * (N - 1))
    return np.float32(loss)


def _run(embeddings, targets, trace=False, **kw):
    nc = _get_nc()
    in_maps = _make_in_maps(embeddings, targets)
    res = run_bass_kernel_spmd(nc, in_maps, list(range(NC)), trace=trace, **kw)
    outs = [res.results[c] for c in range(NC)]
    return _reduce_outs(outs), res


def kernel(embeddings, targets):
    loss, _ = _run(embeddings, targets, trace=False)
    return loss


# revision 24
# speedup vs baseline: 1.1326x; 1.1326x over previous
"""Trainium2 Bass kernel for nn_ContrastiveLoss (N=384, D=128, 8 cores).

Label-structure reduction (exact re-grouping of the reference math):
  y = concat(targets, targets) has every label twice (rows i and i+192).
  Pairwise label distance a[i,j] = |y_i - y_j| therefore lives on a 192x192
  label grid; the per-anchor comparison mask cp[v,P] = [a_v < a_P] depends
  only on the anchor's label row. Columns p and p+192 of denom are equal, and
  j and j+192 contributions pair-reduce. With
     w[i,j] = exp(-dist(z_i,z_j)/TEMP) * sigmoid(TAU*a) * [j != i]
     m[i,v] = coef[v,i] * (w[i,j=v] + w[i,j=v+192]),
     coef = (POS_W-1)*[t_v > y_i] - NEG_W*[t_v <= y_i]
     base_i = sum_j w[i,j]
  denomL[i,P] = base_i + sum_v cp_i[v,P] * m[i,v]        (192 P columns)
  sum_{p!=i} log denom[i,p] = 2*sum_P log denomL[i,P] - log base_i
  loss = -(sum_i sum_{p!=i} s[i,p] - sum_i sum_{p!=i} log denom) / (N(N-1)).

Device work per core (24 label rows = 48 anchor rows):
  phase 1: Gram matmuls -> dist -> exp (ln/exp chain, one ACT table set),
  w tiles, pair-reduce, m weights (bf16, zero-padded to M=32 col groups).
  phase 2: per PSUM bank, 4 col-tiled (tile_position) matmul groups compute
  denomL for 4 row-pairs directly in PSUM (cp ships as host fp8 with a
  bias row carrying base_i), then ONE scalar Ln(+eps, accum_out) per bank
  fuses log + the P-reduction. cp is label-only data, precomputed on host.
"""

import os
import sys

import numpy as np

for _p in ("/opt/trn_rl_repo", "/root/.axon_site/_ro/trn_rl_repo"):
    if os.path.isdir(_p) and _p not in sys.path:
        sys.path.insert(0, _p)

import ml_dtypes

import concourse.bass as bass
import concourse.bacc as bacc
import concourse.mybir as mybir
from concourse import tile
from concourse.bass_utils import run_bass_kernel_spmd

F32 = mybir.dt.float32
BF16 = mybir.dt.bfloat16
FP8 = mybir.dt.float8e4
AF = mybir.ActivationFunctionType
OP = mybir.AluOpType

B = 192
N = 2 * B
D = 128
NC = 8
V = 192           # labels
NP = 24           # label rows (pairs) per core
RW = 2 * NP       # 48 anchor rows per core
PB = 192          # P columns per pair block
NB = 3            # PSUM banks in phase 2 (8 pairs each: 4 col groups x 2 col sets)
NG = 4            # col groups per bank

TEMP = 2.0
TAU = 1.0
POS_W = 0.1
NEG_W = 1.0

# packed_bf column layout (bf16, [128, PW])
C_ZT2 = 0          # 0:384    -2*z~ transposed (j natural order)
C_ZOWN = 384       # 384:432  z~ own cols (2p -> row 24k+p, 2p+1 -> +192)
C_ZOWNX = 432      # 432:480  -2*z~ own cols
C_DWND = 480       # 480:672  dwnd per chunk slice (48 cols each: ch0,ch2,ch1,ch3)
C_COEF = 672       # 672:768  coef0 (128 rows) | coef1 (64 rows)
PW = 768

# j chunks: (z-col slice, Pc, SQ/DWND col slice index)
CHUNKS = [
    (0, 128, 0),      # ch0: j 0:128     v 0:128   copy 1
    (192, 128, 1),    # ch2: j 192:320   v 0:128   copy 2
    (128, 64, 2),     # ch1: j 128:192   v 128:192 copy 1
    (320, 64, 3),     # ch3: j 320:384   v 128:192 copy 2
]


def _build_program():
    # Force the single ACT table set containing Ln+Exp+Copy (set 6,
    # natural_log_exp_and_others): the greedy table chooser otherwise picks
    # exp_and_others for Exp and natural_log for Ln, thrashing 4 loads
    # (1.5us each). Emptying every other set makes set 6 the unique choice;
    # ids stay aligned with act_info.json. Patch is scoped to this build.
    import concourse.bacc as _bacc_mod
    _orig_gat = _bacc_mod.get_activation_tables

    def _gat(arch):
        tables = _orig_gat(arch)
        return {
            name: (funcs if name == "natural_log_exp_and_others" else set())
            for name, funcs in tables.items()
        }

    _bacc_mod.get_activation_tables = _gat
    try:
        return _build_program_inner()
    finally:
        _bacc_mod.get_activation_tables = _orig_gat


def _build_program_inner():
    nc = bacc.Bacc("TRN2", target_bir_lowering=False, debug=False, num_devices=NC)

    packed = nc.dram_tensor("packed", [128, PW], BF16, kind="ExternalInput").ap()
    cpa_d = nc.dram_tensor("cpa", [128, NP * PB], FP8, kind="ExternalInput").ap()
    cpb_d = nc.dram_tensor("cpb", [65, NP * PB], FP8, kind="ExternalInput").ap()
    out_acc = nc.dram_tensor("acc", [1, 2 * RW + NB], F32, kind="ExternalOutput").ap()

    with tile.TileContext(nc) as tc:
        with (
            tc.tile_pool(name="sb", bufs=1) as sb,
            tc.tile_pool(name="lnp", bufs=2) as lnp,
        ):
            # ---------------- DMAs ----------------
            # packed in two regions so the Gram/zsq work starts before the
            # dwnd/coef half lands
            pk = sb.tile([128, PW], BF16, tag="pk")
            nc.sync.dma_start(pk[:, 0:C_DWND], packed[:, 0:C_DWND])
            nc.scalar.dma_start(pk[:, C_DWND:PW], packed[:, C_DWND:PW])
            cpa = sb.tile([128, NP * PB], FP8, tag="cpa")
            cpb = sb.tile([65, NP * PB], FP8, tag="cpb")
            for b in range(NB):
                sl = slice(b * 8 * PB, (b + 1) * 8 * PB)
                eng = nc.sync if b % 2 == 0 else nc.scalar
                eng.dma_start(cpa[:, sl], cpa_d[:, sl])
                eng2 = nc.scalar if b % 2 == 0 else nc.sync
                eng2.dma_start(cpb[:, sl], cpb_d[:, sl])

            # ---------------- consts ----------------
            quart = sb.tile([128, 1], BF16, tag="quart")
            nc.vector.memset(quart[:], 0.25)
            ones_b = sb.tile([128, 1], BF16, tag="ones_b")
            nc.vector.memset(ones_b[:], 1.0)
            ones_f = sb.tile([128, 1], F32, tag="ones_f")
            nc.vector.memset(ones_f[:], 1.0)
            onesrow = sb.tile([1, 128], BF16, tag="onesrow")
            nc.vector.memset(onesrow[:], 1.0)
            epscol = sb.tile([128, 1], F32, tag="eps")
            nc.vector.memset(epscol[:], 1e-30)
            vmask = sb.tile([128, 1], F32, tag="vmask")
            nc.vector.memset(vmask[:], 0.0)
            for q in range(NG):
                nc.vector.memset(vmask[32 * q : 32 * q + 2, :], 1.0)
            scr = sb.tile([128, 1], F32, tag="scr")
            # first scalar op: force-load the ln/exp ACT table set early
            nc.scalar.activation(scr[:], epscol[:], AF.Exp)

            mA = sb.tile([128, NP * 32], BF16, tag="mA")
            nc.gpsimd.memset(mA[:], 0.0)
            mB = sb.tile([65, NP * 32], BF16, tag="mB")
            nc.gpsimd.memset(mB[:], 0.0)
            # sqm pre-filled with 1.0: rows 64:128 of the 64-partition chunk
            # slices stay 1.0 -> ln 0, dist 1, wnd = e*0 = 0 (no NaNs)
            sqm = sb.tile([128, 4 * RW], F32, tag="sqm")
            nc.gpsimd.memset(sqm[:], 1.0)

            # ---------------- phase 1 ----------------
            zsq4 = sb.tile([128, N], BF16, tag="zsq4")
            nc.vector.tensor_tensor(
                zsq4[:], pk[:, C_ZT2 : C_ZT2 + N], pk[:, C_ZT2 : C_ZT2 + N],
                op=OP.mult,
            )
            zsqo4 = sb.tile([128, RW], BF16, tag="zsqo4")
            nc.gpsimd.tensor_tensor(
                zsqo4[:], pk[:, C_ZOWNX : C_ZOWNX + RW],
                pk[:, C_ZOWNX : C_ZOWNX + RW], op=OP.mult,
            )

            with (
                tc.tile_pool(name="ps_own", bufs=1, space="PSUM") as ps_own,
                tc.tile_pool(name="ps_g", bufs=1, space="PSUM") as ps_g,
                tc.tile_pool(name="ps_acc", bufs=1, space="PSUM") as ps_acc,
                tc.tile_pool(name="ps_acd", bufs=1, space="PSUM") as ps_acd,
            ):
                # n2own row [1,48] and n2col row [1,384] (0.25 * colsums of 4z^2)
                n2own_ps = ps_own.tile([1, RW], F32, tag="own")
                nc.tensor.matmul(
                    n2own_ps[:], quart[:], zsqo4[:], start=True, stop=True
                )
                n2row_ps = ps_own.tile([1, N], F32, tag="n2r")
                for zc, pc in ((0, 128), (192, 128), (128, 64), (320, 64)):
                    nc.tensor.matmul(
                        n2row_ps[0:1, zc : zc + pc], quart[:],
                        zsq4[:, zc : zc + pc], start=True, stop=True,
                        skip_group_check=True,
                    )
                n2own_s = sb.tile([1, RW], BF16, tag="n2own_s")
                nc.vector.tensor_copy(n2own_s[:], n2own_ps[:])
                n2row_s = sb.tile([1, N], BF16, tag="n2row_s")
                for zc, pc in ((0, 128), (192, 128), (128, 64), (320, 64)):
                    nc.vector.tensor_copy(
                        n2row_s[0:1, zc : zc + pc], n2row_ps[0:1, zc : zc + pc]
                    )

                lsq = sb.tile([128, 4 * RW], F32, tag="lsq")
                dist = sb.tile([128, 4 * RW], F32, tag="dist")
                e_t = sb.tile([128, 4 * RW], BF16, tag="e")
                wnd = sb.tile([128, 4 * RW], BF16, tag="wnd")
                acc_ps = ps_acc.tile([1, RW], F32, tag="acc")
                acd_ps = ps_acd.tile([1, RW], F32, tag="acd")

                # halves: A = ch0+ch2 (128 rows), B = ch1+ch3 (64 rows)
                HALVES = [(0, 128, (0, 192)), (1, 64, (128, 320))]
                acc_first = True
                for hi, pc, zcs in HALVES:
                    hsl = slice(hi * 2 * RW, (hi + 1) * 2 * RW)
                    gb = ps_g.tile([pc, 2 * RW], F32, tag=f"gb{hi}")
                    for ci, zc in enumerate(zcs):
                        csl = slice(ci * RW, (ci + 1) * RW)
                        nc.tensor.matmul(
                            gb[:, csl], pk[:, C_ZT2 + zc : C_ZT2 + zc + pc],
                            pk[:, C_ZOWN : C_ZOWN + RW],
                            start=(ci == 0), stop=False,
                            skip_group_check=True,
                        )
                    for ci, zc in enumerate(zcs):
                        csl = slice(ci * RW, (ci + 1) * RW)
                        nc.tensor.matmul(
                            gb[:, csl], onesrow[0:1, 0:pc], n2own_s[:],
                            start=False, stop=False, skip_group_check=True,
                        )
                        nc.tensor.matmul(
                            gb[:, csl], n2row_s[0:1, zc : zc + pc],
                            onesrow[0:1, 0:RW],
                            start=False, stop=(ci == 1), skip_group_check=True,
                        )
                    nc.vector.tensor_scalar(
                        sqm[0:pc, hsl], gb[:], 0.0, None, op0=OP.max
                    )
                    nc.scalar.activation(
                        lsq[:, hsl], sqm[:, hsl], AF.Ln, bias=epscol[:]
                    )
                    nc.scalar.activation(
                        dist[:, hsl], lsq[:, hsl], AF.Exp, scale=0.5
                    )
                    nc.scalar.activation(
                        e_t[:, hsl], dist[:, hsl], AF.Exp, scale=-1.0 / TEMP
                    )
                    nc.vector.tensor_tensor(
                        wnd[:, hsl], e_t[:, hsl],
                        pk[:, C_DWND + hi * 2 * RW : C_DWND + (hi + 1) * 2 * RW],
                        op=OP.mult,
                    )
                    # column sums for this half (2 wnd + 2 dist chunk slices)
                    for ci in range(2):
                        sl2 = slice((2 * hi + ci) * RW, (2 * hi + ci + 1) * RW)
                        nc.tensor.matmul(
                            acc_ps[0:1, 0:RW], ones_b[0:pc, :], wnd[0:pc, sl2],
                            start=(hi == 0 and ci == 0),
                            stop=(hi == 1 and ci == 1),
                            skip_group_check=True,
                        )
                    for ci in range(2):
                        sl2 = slice((2 * hi + ci) * RW, (2 * hi + ci + 1) * RW)
                        nc.tensor.matmul(
                            acd_ps[0:1, 0:RW], ones_f[0:pc, :],
                            dist[0:pc, sl2],
                            start=(hi == 0 and ci == 0),
                            stop=(hi == 1 and ci == 1),
                            skip_group_check=True,
                        )
                    # pair-reduce + m weights for this half
                    if hi == 0:
                        wr0 = sb.tile([128, RW], BF16, tag="wr0")
                        nc.vector.tensor_tensor(
                            wr0[:], wnd[:, 0:RW], wnd[:, RW : 2 * RW], op=OP.add
                        )
                        mA_sc = mA[:].rearrange(
                            "p (n s) -> p n s", n=NP, s=32
                        )[:, :, 0:2]
                        nc.vector.tensor_tensor(
                            mA_sc,
                            wr0[:].rearrange("p (n r) -> p n r", n=NP, r=2),
                            pk[:, C_COEF : C_COEF + RW].rearrange(
                                "p (n r) -> p n r", n=NP, r=2
                            ),
                            op=OP.mult,
                        )
                    else:
                        wr1 = sb.tile([64, RW], BF16, tag="wr1")
                        nc.vector.tensor_tensor(
                            wr1[:], wnd[0:64, 2 * RW : 3 * RW],
                            wnd[0:64, 3 * RW : 4 * RW], op=OP.add
                        )
                        mB_sc = mB[0:64].rearrange(
                            "p (n s) -> p n s", n=NP, s=32
                        )[:, :, 0:2]
                        nc.vector.tensor_tensor(
                            mB_sc,
                            wr1[:].rearrange("p (n r) -> p n r", n=NP, r=2),
                            pk[0:64, C_COEF + RW : C_COEF + 2 * RW].rearrange(
                                "p (n r) -> p n r", n=NP, r=2
                            ),
                            op=OP.mult,
                        )

                # bias row straight from PSUM on the vector engine
                mBias_sc = mB[64:65].rearrange(
                    "p (n s) -> p n s", n=NP, s=32
                )[:, :, 0:2]
                nc.vector.tensor_copy(
                    mBias_sc,
                    acc_ps[0:1, 0:RW].rearrange(
                        "p (n r) -> p n r", n=NP, r=2
                    ),
                )
                accsb = sb.tile([1, 2 * RW + NB], F32, tag="accsb")
                nc.vector.tensor_copy(accsb[0:1, 0:RW], acc_ps[:])
                nc.vector.tensor_copy(accsb[0:1, RW : 2 * RW], acd_ps[:])

            # ---------------- phase 2 ----------------
            red = sb.tile([128, NB], F32, tag="red")
            with tc.tile_pool(name="ps_den", bufs=3, space="PSUM") as ps_den:
                for b in range(NB):
                    den = ps_den.tile([128, 2 * PB], F32, tag="den")
                    for cs in range(2):
                        for q in range(NG):
                            p = 8 * b + 4 * cs + q
                            osl = den[
                                32 * q : 32 * q + 32, PB * cs : PB * (cs + 1)
                            ]
                            nc.tensor.matmul(
                                osl,
                                mA[:, 32 * p : 32 * p + 32],
                                cpa[:, PB * p : PB * (p + 1)],
                                start=(cs == 0), stop=False,
                                skip_group_check=True,
                                tile_position=(0, 32 * q),
                            )
                        for q in range(NG):
                            p = 8 * b + 4 * cs + q
                            osl = den[
                                32 * q : 32 * q + 32, PB * cs : PB * (cs + 1)
                            ]
                            nc.tensor.matmul(
                                osl,
                                mB[:, 32 * p : 32 * p + 32],
                                cpb[:, PB * p : PB * (p + 1)],
                                start=False, stop=(cs == 1),
                                skip_group_check=True,
                                tile_position=(0, 32 * q),
                            )
                    lnden = lnp.tile([128, 2 * PB], BF16, tag="ln")
                    nc.scalar.activation(
                        lnden[:], den[:], AF.Ln, bias=epscol[:],
                        accum_out=red[:, b : b + 1],
                    )
                # collapse the sparse red columns to 6 totals on the PE and
                # append to the single-row output (one DMA descriptor)
                redtot_ps = ps_den.tile([1, NB], F32, tag="rt")
                nc.tensor.matmul(
                    redtot_ps[:], vmask[:], red[:], start=True, stop=True
                )
                nc.vector.tensor_copy(accsb[0:1, 2 * RW :], redtot_ps[:])
            nc.sync.dma_start(out_acc, accsb[:])

    nc.compile()
    return nc


_NC_CACHE = None


def _get_nc():
    global _NC_CACHE
    if _NC_CACHE is None:
        _NC_CACHE = _build_program()
    return _NC_CACHE


def _make_in_maps(embeddings, targets):
    emb = np.ascontiguousarray(np.asarray(embeddings, dtype=np.float32))
    tgt = np.ascontiguousarray(np.asarray(targets, dtype=np.float32))
    z = emb.transpose(1, 0, 2).reshape(N, D)
    zb = z.astype(ml_dtypes.bfloat16)              # device z values
    t = tgt[:, 0]                                  # 192 labels (fp32)
    y = np.concatenate([t, t])                     # 384

    in_maps = []
    for core in range(NC):
        labs = np.arange(NP * core, NP * (core + 1))       # label rows
        own = np.empty(RW, np.int64)
        own[0::2] = labs
        own[1::2] = labs + B
        yo = y[own]                                        # [48]

        pkt = np.zeros((128, PW), np.float32)
        pkt[:, C_ZT2 : C_ZT2 + N] = -2.0 * zb.T.astype(np.float32)
        pkt[:, C_ZOWN : C_ZOWN + RW] = zb.T[:, own].astype(np.float32)
        pkt[:, C_ZOWNX : C_ZOWNX + RW] = -2.0 * zb.T[:, own].astype(np.float32)

        # dwnd = sigmoid(TAU*|y_i - y_j|) * [j != own_row], chunk layout
        a_all = np.abs(yo[None, :] - y[:, None])           # [384, 48] (j, i)
        dw = 1.0 / (1.0 + np.exp(-TAU * a_all))
        ndm = (np.arange(N)[:, None] != own[None, :]).astype(np.float32)
        dwnd = (dw * ndm).astype(np.float32)
        for (zc, pc, si) in CHUNKS:
            pkt[0:pc, C_DWND + si * RW : C_DWND + (si + 1) * RW] = dwnd[
                zc : zc + pc, :
            ]

        # coef[v, i] = (POS_W-1) if t_v > y_i else -NEG_W
        same = (t[:, None] > yo[None, :])
        coef = np.where(same, POS_W - 1.0, -NEG_W).astype(np.float32)
        pkt[:, C_COEF : C_COEF + RW] = coef[0:128]
        pkt[0:64, C_COEF + RW : C_COEF + 2 * RW] = coef[128:192]

        # cp blocks (exact fp32 label comparisons)
        cpa = np.zeros((128, NP * PB), np.float32)
        cpb = np.zeros((65, NP * PB), np.float32)
        for p, L in enumerate(labs):
            al = np.abs(t[L] - t)                          # [192]
            cp = (al[:, None] < al[None, :]).astype(np.float32)
            cpa[:, PB * p : PB * (p + 1)] = cp[0:128]
            cpb[0:64, PB * p : PB * (p + 1)] = cp[128:192]
        cpb[64, :] = 1.0

        in_maps.append({
            "packed": pkt.astype(ml_dtypes.bfloat16),
            "cpa": cpa.astype(ml_dtypes.float8_e4m3fn),
            "cpb": cpb.astype(ml_dtypes.float8_e4m3fn),
        })
    return in_maps


def _reduce_outs(outs_list):
    tot_s = 0.0
    tot_logd = 0.0
    for o in outs_list:
        acc = np.asarray(o["acc"], dtype=np.float64)[0]    # [99]
        base = acc[0:RW]
        dsum = acc[RW : 2 * RW]
        redtot = acc[2 * RW : 2 * RW + NB]
        tot_s += -dsum.sum() / TEMP
        tot_logd += 2.0 * redtot.sum() - np.log(base).sum()
    loss = -(tot_s - tot_logd) / (N * (N - 1))
    return np.float32(loss)


def _run(embeddings, targets, trace=False, **kw):
    nc = _get_nc()
    in_maps = _make_in_maps(embeddings, targets)
    res = run_bass_kernel_spmd(nc, in_maps, list(range(NC)), trace=trace, **kw)
    outs = [res.results[c] for c in range(NC)]
    return _reduce_outs(outs), res


def kernel(embeddings, targets):
    loss, _ = _run(embeddings, targets, trace=False)
    return loss


# revision 25
# speedup vs baseline: 1.1832x; 1.0447x over previous
"""Trainium2 Bass kernel for nn_ContrastiveLoss (N=384, D=128, 8 cores).

Label-structure reduction (exact re-grouping of the reference math):
  y = concat(targets, targets) has every label twice (rows i and i+192).
  Pairwise label distance a[i,j] = |y_i - y_j| therefore lives on a 192x192
  label grid; the per-anchor comparison mask cp[v,P] = [a_v < a_P] depends
  only on the anchor's label row. Columns p and p+192 of denom are equal, and
  j and j+192 contributions pair-reduce. With
     w[i,j] = exp(-dist(z_i,z_j)/TEMP) * sigmoid(TAU*a) * [j != i]
     m[i,v] = coef[v,i] * (w[i,j=v] + w[i,j=v+192]),
     coef = (POS_W-1)*[t_v > y_i] - NEG_W*[t_v <= y_i]
     base_i = sum_j w[i,j]
  denomL[i,P] = base_i + sum_v cp_i[v,P] * m[i,v]        (192 P columns)
  sum_{p!=i} log denom[i,p] = 2*sum_P log denomL[i,P] - log base_i
  loss = -(sum_i sum_{p!=i} s[i,p] - sum_i sum_{p!=i} log denom) / (N(N-1)).

Device work per core (24 label rows = 48 anchor rows):
  phase 1: Gram matmuls -> dist -> exp (ln/exp chain, one ACT table set),
  w tiles, pair-reduce, m weights (bf16, zero-padded to M=32 col groups).
  phase 2: per PSUM bank, 4 col-tiled (tile_position) matmul groups compute
  denomL for 4 row-pairs directly in PSUM (cp ships as host fp8 with a
  bias row carrying base_i), then ONE scalar Ln(+eps, accum_out) per bank
  fuses log + the P-reduction. cp is label-only data, precomputed on host.
"""

import os
import sys

import numpy as np

for _p in ("/opt/trn_rl_repo", "/root/.axon_site/_ro/trn_rl_repo"):
    if os.path.isdir(_p) and _p not in sys.path:
        sys.path.insert(0, _p)

import ml_dtypes

import concourse.bass as bass
import concourse.bacc as bacc
import concourse.mybir as mybir
from concourse import tile
from concourse.bass_utils import run_bass_kernel_spmd

F32 = mybir.dt.float32
BF16 = mybir.dt.bfloat16
FP8 = mybir.dt.float8e4
AF = mybir.ActivationFunctionType
OP = mybir.AluOpType

B = 192
N = 2 * B
D = 128
NC = 8
V = 192           # labels
NP = 24           # label rows (pairs) per core
RW = 2 * NP       # 48 anchor rows per core
PB = 192          # P columns per pair block
NB = 3            # PSUM banks in phase 2 (8 pairs each: 4 col groups x 2 col sets)
NG = 4            # col groups per bank

TEMP = 2.0
TAU = 1.0
POS_W = 0.1
NEG_W = 1.0

# packed_bf column layout (bf16, [128, PW])
C_ZT2 = 0          # 0:384    -2*z~ transposed (j natural order)
C_ZOWN = 384       # 384:432  z~ own cols (2p -> row 24k+p, 2p+1 -> +192)
C_ZOWNX = 432      # 432:480  -2*z~ own cols
C_DWND = 480       # 480:672  dwnd per chunk slice (48 cols each: ch0,ch2,ch1,ch3)
C_COEF = 672       # 672:768  coef0 (128 rows) | coef1 (64 rows)
PW = 768

# j chunks: (z-col slice, Pc, SQ/DWND col slice index)
CHUNKS = [
    (0, 128, 0),      # ch0: j 0:128     v 0:128   copy 1
    (192, 128, 1),    # ch2: j 192:320   v 0:128   copy 2
    (128, 64, 2),     # ch1: j 128:192   v 128:192 copy 1
    (320, 64, 3),     # ch3: j 320:384   v 128:192 copy 2
]


def _build_program():
    # Force the single ACT table set containing Ln+Exp+Copy (set 6,
    # natural_log_exp_and_others): the greedy table chooser otherwise picks
    # exp_and_others for Exp and natural_log for Ln, thrashing 4 loads
    # (1.5us each). Emptying every other set makes set 6 the unique choice;
    # ids stay aligned with act_info.json. Patch is scoped to this build.
    import concourse.bacc as _bacc_mod
    _orig_gat = _bacc_mod.get_activation_tables

    def _gat(arch):
        tables = _orig_gat(arch)
        return {
            name: (funcs if name == "natural_log_exp_and_others" else set())
            for name, funcs in tables.items()
        }

    _bacc_mod.get_activation_tables = _gat
    try:
        return _build_program_inner()
    finally:
        _bacc_mod.get_activation_tables = _orig_gat


def _build_program_inner():
    nc = bacc.Bacc("TRN2", target_bir_lowering=False, debug=False, num_devices=NC)

    packed = nc.dram_tensor("packed", [128, PW], BF16, kind="ExternalInput").ap()
    cpa_d = nc.dram_tensor("cpa", [128, NP * PB], FP8, kind="ExternalInput").ap()
    cpb_d = nc.dram_tensor("cpb", [65, NP * PB], FP8, kind="ExternalInput").ap()
    out_acc = nc.dram_tensor("acc", [1, 2 * RW + NB], F32, kind="ExternalOutput").ap()

    with tile.TileContext(nc) as tc:
        with (
            tc.tile_pool(name="sb", bufs=1) as sb,
            tc.tile_pool(name="lnp", bufs=2) as lnp,
        ):
            # ---------------- DMAs ----------------
            # packed in two regions so the Gram/zsq work starts before the
            # dwnd/coef half lands
            pk = sb.tile([128, PW], BF16, tag="pk")
            nc.sync.dma_start(pk[:, 0:C_DWND], packed[:, 0:C_DWND])
            nc.scalar.dma_start(pk[:, C_DWND:PW], packed[:, C_DWND:PW])
            cpa = sb.tile([128, NP * PB], FP8, tag="cpa")
            cpb = sb.tile([65, NP * PB], FP8, tag="cpb")
            for b in range(NB):
                sl = slice(b * 8 * PB, (b + 1) * 8 * PB)
                eng = nc.sync if b % 2 == 0 else nc.scalar
                eng.dma_start(cpa[:, sl], cpa_d[:, sl])
                eng2 = nc.scalar if b % 2 == 0 else nc.sync
                eng2.dma_start(cpb[:, sl], cpb_d[:, sl])

            # ---------------- consts ----------------
            quart = sb.tile([128, 1], BF16, tag="quart")
            nc.vector.memset(quart[:], 0.25)
            ones_b = sb.tile([128, 1], BF16, tag="ones_b")
            nc.vector.memset(ones_b[:], 1.0)
            ones_f = sb.tile([128, 1], F32, tag="ones_f")
            nc.vector.memset(ones_f[:], 1.0)
            onesrow = sb.tile([1, 128], BF16, tag="onesrow")
            nc.vector.memset(onesrow[:], 1.0)
            epscol = sb.tile([128, 1], F32, tag="eps")
            nc.vector.memset(epscol[:], 1e-30)
            vmask = sb.tile([128, 1], F32, tag="vmask")
            nc.vector.memset(vmask[:], 0.0)
            for q in range(NG):
                nc.vector.memset(vmask[32 * q : 32 * q + 2, :], 1.0)
            scr = sb.tile([128, 1], F32, tag="scr")
            # first scalar op: force-load the ln/exp ACT table set early
            nc.scalar.activation(scr[:], epscol[:], AF.Exp)

            mA = sb.tile([128, NP * 32], BF16, tag="mA")
            nc.gpsimd.memset(mA[:], 0.0)
            mB = sb.tile([65, NP * 32], BF16, tag="mB")
            nc.gpsimd.memset(mB[:], 0.0)
            # sqm pre-filled with 1.0: rows 64:128 of the 64-partition chunk
            # slices stay 1.0 -> ln 0, dist 1, wnd = e*0 = 0 (no NaNs)
            sqm = sb.tile([128, 4 * RW], F32, tag="sqm")
            nc.gpsimd.memset(sqm[:], 1.0)

            # ---------------- phase 1 ----------------
            zsq4 = sb.tile([128, N], BF16, tag="zsq4")
            nc.vector.tensor_tensor(
                zsq4[:], pk[:, C_ZT2 : C_ZT2 + N], pk[:, C_ZT2 : C_ZT2 + N],
                op=OP.mult,
            )
            zsqo4 = sb.tile([128, RW], BF16, tag="zsqo4")
            nc.gpsimd.tensor_tensor(
                zsqo4[:], pk[:, C_ZOWNX : C_ZOWNX + RW],
                pk[:, C_ZOWNX : C_ZOWNX + RW], op=OP.mult,
            )

            with (
                tc.tile_pool(name="ps_own", bufs=1, space="PSUM") as ps_own,
                tc.tile_pool(name="ps_g", bufs=1, space="PSUM") as ps_g,
                tc.tile_pool(name="ps_acc", bufs=1, space="PSUM") as ps_acc,
                tc.tile_pool(name="ps_acd", bufs=1, space="PSUM") as ps_acd,
            ):
                # n2own row [1,48] and n2col row [1,384] (0.25 * colsums of 4z^2)
                n2own_ps = ps_own.tile([1, RW], F32, tag="own")
                nc.tensor.matmul(
                    n2own_ps[:], quart[:], zsqo4[:], start=True, stop=True
                )
                n2own_s = sb.tile([1, RW], BF16, tag="n2own_s")
                nc.vector.tensor_copy(n2own_s[:], n2own_ps[:])
                # per-chunk n2col columns, all in one PSUM bank
                n2all = ps_own.tile([128, 4], F32, tag="n2c")
                for ci, (zc, pc) in enumerate(
                    ((0, 128), (192, 128), (128, 64), (320, 64))
                ):
                    nc.tensor.matmul(
                        n2all[0:pc, ci : ci + 1], zsq4[:, zc : zc + pc],
                        quart[:], start=True, stop=True,
                        skip_group_check=True,
                    )

                lsq = sb.tile([128, 4 * RW], F32, tag="lsq")
                dist = sb.tile([128, 4 * RW], F32, tag="dist")
                e_t = sb.tile([128, 4 * RW], BF16, tag="e")
                wnd = sb.tile([128, 4 * RW], BF16, tag="wnd")
                acc_ps = ps_acc.tile([1, RW], F32, tag="acc")
                acd_ps = ps_acd.tile([1, RW], F32, tag="acd")

                # halves: A = ch0+ch2 (128 rows), B = ch1+ch3 (64 rows)
                HALVES = [(0, 128, (0, 192)), (1, 64, (128, 320))]
                acc_first = True
                for hi, pc, zcs in HALVES:
                    hsl = slice(hi * 2 * RW, (hi + 1) * 2 * RW)
                    gb = ps_g.tile([pc, 2 * RW], F32, tag=f"gb{hi}")
                    for ci, zc in enumerate(zcs):
                        csl = slice(ci * RW, (ci + 1) * RW)
                        nc.tensor.matmul(
                            gb[:, csl], pk[:, C_ZT2 + zc : C_ZT2 + zc + pc],
                            pk[:, C_ZOWN : C_ZOWN + RW],
                            start=(ci == 0), stop=False,
                            skip_group_check=True,
                        )
                    for ci, zc in enumerate(zcs):
                        csl = slice(ci * RW, (ci + 1) * RW)
                        nc.tensor.matmul(
                            gb[:, csl], onesrow[0:1, 0:pc], n2own_s[:],
                            start=False, stop=(ci == 1), skip_group_check=True,
                        )
                    for ci in range(2):
                        csl = slice(ci * RW, (ci + 1) * RW)
                        nc.vector.tensor_scalar(
                            sqm[0:pc, (2 * hi + ci) * RW : (2 * hi + ci + 1) * RW],
                            gb[:, csl], n2all[0:pc, 2 * hi + ci : 2 * hi + ci + 1],
                            0.0, op0=OP.add, op1=OP.max,
                        )
                    nc.scalar.activation(
                        lsq[:, hsl], sqm[:, hsl], AF.Ln, bias=epscol[:]
                    )
                    nc.scalar.activation(
                        dist[:, hsl], lsq[:, hsl], AF.Exp, scale=0.5
                    )
                    nc.scalar.activation(
                        e_t[:, hsl], dist[:, hsl], AF.Exp, scale=-1.0 / TEMP
                    )
                    nc.vector.tensor_tensor(
                        wnd[:, hsl], e_t[:, hsl],
                        pk[:, C_DWND + hi * 2 * RW : C_DWND + (hi + 1) * 2 * RW],
                        op=OP.mult,
                    )
                    # column sums for this half (2 wnd + 2 dist chunk slices)
                    for ci in range(2):
                        sl2 = slice((2 * hi + ci) * RW, (2 * hi + ci + 1) * RW)
                        nc.tensor.matmul(
                            acc_ps[0:1, 0:RW], ones_b[0:pc, :], wnd[0:pc, sl2],
                            start=(hi == 0 and ci == 0),
                            stop=(hi == 1 and ci == 1),
                            skip_group_check=True,
                        )
                    for ci in range(2):
                        sl2 = slice((2 * hi + ci) * RW, (2 * hi + ci + 1) * RW)
                        nc.tensor.matmul(
                            acd_ps[0:1, 0:RW], ones_f[0:pc, :],
                            dist[0:pc, sl2],
                            start=(hi == 0 and ci == 0),
                            stop=(hi == 1 and ci == 1),
                            skip_group_check=True,
                        )
                    # pair-reduce + m weights for this half
                    if hi == 0:
                        wr0 = sb.tile([128, RW], BF16, tag="wr0")
                        nc.vector.tensor_tensor(
                            wr0[:], wnd[:, 0:RW], wnd[:, RW : 2 * RW], op=OP.add
                        )
                        mA_sc = mA[:].rearrange(
                            "p (n s) -> p n s", n=NP, s=32
                        )[:, :, 0:2]
                        nc.vector.tensor_tensor(
                            mA_sc,
                            wr0[:].rearrange("p (n r) -> p n r", n=NP, r=2),
                            pk[:, C_COEF : C_COEF + RW].rearrange(
                                "p (n r) -> p n r", n=NP, r=2
                            ),
                            op=OP.mult,
                        )
                    else:
                        wr1 = sb.tile([64, RW], BF16, tag="wr1")
                        nc.vector.tensor_tensor(
                            wr1[:], wnd[0:64, 2 * RW : 3 * RW],
                            wnd[0:64, 3 * RW : 4 * RW], op=OP.add
                        )
                        mB_sc = mB[0:64].rearrange(
                            "p (n s) -> p n s", n=NP, s=32
                        )[:, :, 0:2]
                        nc.vector.tensor_tensor(
                            mB_sc,
                            wr1[:].rearrange("p (n r) -> p n r", n=NP, r=2),
                            pk[0:64, C_COEF + RW : C_COEF + 2 * RW].rearrange(
                                "p (n r) -> p n r", n=NP, r=2
                            ),
                            op=OP.mult,
                        )

                # bias row straight from PSUM on the vector engine
                mBias_sc = mB[64:65].rearrange(
                    "p (n s) -> p n s", n=NP, s=32
                )[:, :, 0:2]
                nc.vector.tensor_copy(
                    mBias_sc,
                    acc_ps[0:1, 0:RW].rearrange(
                        "p (n r) -> p n r", n=NP, r=2
                    ),
                )
                accsb = sb.tile([1, 2 * RW + NB], F32, tag="accsb")
                nc.vector.tensor_copy(accsb[0:1, 0:RW], acc_ps[:])
                nc.vector.tensor_copy(accsb[0:1, RW : 2 * RW], acd_ps[:])

            # ---------------- phase 2 ----------------
            red = sb.tile([128, NB], F32, tag="red")
            with tc.tile_pool(name="ps_den", bufs=3, space="PSUM") as ps_den:
                for b in range(NB):
                    den = ps_den.tile([128, 2 * PB], F32, tag="den")
                    for cs in range(2):
                        for q in range(NG):
                            p = 8 * b + 4 * cs + q
                            osl = den[
                                32 * q : 32 * q + 32, PB * cs : PB * (cs + 1)
                            ]
                            nc.tensor.matmul(
                                osl,
                                mA[:, 32 * p : 32 * p + 32],
                                cpa[:, PB * p : PB * (p + 1)],
                                start=(cs == 0), stop=False,
                                skip_group_check=True,
                                tile_position=(0, 32 * q),
                            )
                        for q in range(NG):
                            p = 8 * b + 4 * cs + q
                            osl = den[
                                32 * q : 32 * q + 32, PB * cs : PB * (cs + 1)
                            ]
                            nc.tensor.matmul(
                                osl,
                                mB[:, 32 * p : 32 * p + 32],
                                cpb[:, PB * p : PB * (p + 1)],
                                start=False, stop=(cs == 1),
                                skip_group_check=True,
                                tile_position=(0, 32 * q),
                            )
                    lnden = lnp.tile([128, 2 * PB], BF16, tag="ln")
                    nc.scalar.activation(
                        lnden[:], den[:], AF.Ln, bias=epscol[:],
                        accum_out=red[:, b : b + 1],
                    )
                # collapse the sparse red columns to 6 totals on the PE and
                # append to the single-row output (one DMA descriptor)
                redtot_ps = ps_den.tile([1, NB], F32, tag="rt")
                nc.tensor.matmul(
                    redtot_ps[:], vmask[:], red[:], start=True, stop=True
                )
                nc.vector.tensor_copy(accsb[0:1, 2 * RW :], redtot_ps[:])
            nc.sync.dma_start(out_acc, accsb[:])

    nc.compile()
    return nc


_NC_CACHE = None


def _get_nc():
    global _NC_CACHE
    if _NC_CACHE is None:
        _NC_CACHE = _build_program()
    return _NC_CACHE


def _make_in_maps(embeddings, targets):
    emb = np.ascontiguousarray(np.asarray(embeddings, dtype=np.float32))
    tgt = np.ascontiguousarray(np.asarray(targets, dtype=np.float32))
    z = emb.transpose(1, 0, 2).reshape(N, D)
    zb = z.astype(ml_dtypes.bfloat16)              # device z values
    t = tgt[:, 0]                                  # 192 labels (fp32)
    y = np.concatenate([t, t])                     # 384

    in_maps = []
    for core in range(NC):
        labs = np.arange(NP * core, NP * (core + 1))       # label rows
        own = np.empty(RW, np.int64)
        own[0::2] = labs
        own[1::2] = labs + B
        yo = y[own]                                        # [48]

        pkt = np.zeros((128, PW), np.float32)
        pkt[:, C_ZT2 : C_ZT2 + N] = -2.0 * zb.T.astype(np.float32)
        pkt[:, C_ZOWN : C_ZOWN + RW] = zb.T[:, own].astype(np.float32)
        pkt[:, C_ZOWNX : C_ZOWNX + RW] = -2.0 * zb.T[:, own].astype(np.float32)

        # dwnd = sigmoid(TAU*|y_i - y_j|) * [j != own_row], chunk layout
        a_all = np.abs(yo[None, :] - y[:, None])           # [384, 48] (j, i)
        dw = 1.0 / (1.0 + np.exp(-TAU * a_all))
        ndm = (np.arange(N)[:, None] != own[None, :]).astype(np.float32)
        dwnd = (dw * ndm).astype(np.float32)
        for (zc, pc, si) in CHUNKS:
            pkt[0:pc, C_DWND + si * RW : C_DWND + (si + 1) * RW] = dwnd[
                zc : zc + pc, :
            ]

        # coef[v, i] = (POS_W-1) if t_v > y_i else -NEG_W
        same = (t[:, None] > yo[None, :])
        coef = np.where(same, POS_W - 1.0, -NEG_W).astype(np.float32)
        pkt[:, C_COEF : C_COEF + RW] = coef[0:128]
        pkt[0:64, C_COEF + RW : C_COEF + 2 * RW] = coef[128:192]

        # cp blocks (exact fp32 label comparisons)
        cpa = np.zeros((128, NP * PB), np.float32)
        cpb = np.zeros((65, NP * PB), np.float32)
        for p, L in enumerate(labs):
            al = np.abs(t[L] - t)                          # [192]
            cp = (al[:, None] < al[None, :]).astype(np.float32)
            cpa[:, PB * p : PB * (p + 1)] = cp[0:128]
            cpb[0:64, PB * p : PB * (p + 1)] = cp[128:192]
        cpb[64, :] = 1.0

        in_maps.append({
            "packed": pkt.astype(ml_dtypes.bfloat16),
            "cpa": cpa.astype(ml_dtypes.float8_e4m3fn),
            "cpb": cpb.astype(ml_dtypes.float8_e4m3fn),
        })
    return in_maps


def _reduce_outs(outs_list):
    tot_s = 0.0
    tot_logd = 0.0
    for o in outs_list:
        acc = np.asarray(o["acc"], dtype=np.float64)[0]    # [99]
        base = acc[0:RW]
        dsum = acc[RW : 2 * RW]
        redtot = acc[2 * RW : 2 * RW + NB]
        tot_s += -dsum.sum() / TEMP
        tot_logd += 2.0 * redtot.sum() - np.log(base).sum()
    loss = -(tot_s - tot_logd) / (N * (N - 1))
    return np.float32(loss)


def _run(embeddings, targets, trace=False, **kw):
    nc = _get_nc()
    in_maps = _make_in_maps(embeddings, targets)
    res = run_bass_kernel_spmd(nc, in_maps, list(range(NC)), trace=trace, **kw)
    outs = [res.results[c] for c in range(NC)]
    return _reduce_outs(outs), res


def kernel(embeddings, targets):
    loss, _ = _run(embeddings, targets, trace=False)
    return loss
